# revision 41
# baseline (speedup 1.0000x reference)
"""Trainium2 Bass kernel for nn_BatchGRUNet (bidirectional GRU over ragged graph batch).

Contract: kernel(**inputs) takes the FULL unsharded inputs (as produced by
reference.setup_inputs()) and returns the FULL [N+1, 2H] output.

Strategy (8 NeuronCores, SPMD):
  - 2048 graphs are split into 4 shards of 512 graphs.
  - Cores 0..3 run the FORWARD GRU for shards 0..3; cores 4..7 run the
    BACKWARD GRU for the same shards, fed with time-reversed packed input
    (a backward scan == forward scan on reversed time), so all 8 cores run
    the identical single-direction program on different data.
  - Host packs the ragged node rows into a padded, transposed layout
    xpad[h, t, b] (fill -1e30) per core; the device computes
    hpool (segment max), message = relu(node + bias), the 64-step GRU
    recurrence (fp32, matmuls in float32r), and writes padded-transposed
    outputs y[h, t, b]; host unpads/gathers back to flat node order.

Layout on device (per core, per step):
  - state h^T [300, 512] lives H-on-partitions in 3 chunks (128/128/44+aug).
  - i = x@W_ih^T + b_ih + h@W_hh^T + b_hh accumulated per gate directly in
    PSUM (biases folded in via an augmented ones-row on x / h and an extra
    bias row on the weights).
  - r/z: sigmoid straight out of PSUM; n-gate keeps xp and gh parts separate
    (torch GRU: n = tanh(i_n + r * h_n)).
"""

import os

import numpy as np

H = 300
GATE3 = 3 * H  # 900
HCHUNK = [(0, 128), (128, 128), (256, 44)]  # (start, len) partition chunks of H
NEG_FILL = np.float32(-1.0e30)

_BUILD_CACHE = {}


def _build_program(NB, L, no_pack=False, split_sigma=False, plain_dma=False, body_level=4, psum_bufs=2, only_g=None, flat_rhs=False):
    """Build the single-direction GRU Bass program (SPMD across cores).

    NB: per-core batch (number of graphs); L: number of time steps.

    Matmul inputs (x, h, W) are float32r (tf32): the PE streams fp32r at full
    rate when the moving dim >= 256.  Producers round on write (ACT relu for
    x, DVE adds/copies for h and W).  The 44-row K/M tail chunk (H=300 =
    128+128+44) lives at partition base 64 so its xp/gh matmuls can be
    row-packed into disjoint PE quadrants and run concurrently.
    """
    import concourse.bass as bass
    import concourse.tile as tile
    from concourse import bacc, mybir

    f32 = mybir.dt.float32
    F32R = mybir.dt.float32r
    AF = mybir.ActivationFunctionType
    OP = mybir.AluOpType
    X = mybir.AxisListType.X

    nc = bacc.Bacc("TRN2", target_bir_lowering=False)

    # xpad row H (=300) is an all-ones row: the augmented-K input that applies
    # the x-side bias row of wih inside the matmul accumulation.
    xpad_d = nc.dram_tensor("xpad", [H + 1, L, NB], f32, kind="ExternalInput")
    wih_d = nc.dram_tensor("wih", [H + 1, GATE3], f32, kind="ExternalInput")
    whh_d = nc.dram_tensor("whh", [H, GATE3], f32, kind="ExternalInput")
    # biasq cols 0..2: relu bias (node bias) per H-chunk; cols 3..5: b_hh_n chunks
    biasq_d = nc.dram_tensor("biasq", [128, 6], f32, kind="ExternalInput")
    hpool_d = nc.dram_tensor("hpool", [H, NB], f32, kind="ExternalInput")
    y_d = nc.dram_tensor("y", [H, L, NB], f32, kind="ExternalOutput")

    C2 = 44  # tail chunk rows
    B2 = 64  # partition base of the tail chunk (state/weights/gate pipeline)

    with tile.TileContext(nc) as tc:
        with (
            tc.tile_pool(name="singles", bufs=1) as singles,
            tc.tile_pool(name="xpool", bufs=2) as xpool,
            tc.tile_pool(name="ew", bufs=2) as ew,
            tc.tile_pool(name="psum", bufs=psum_bufs, space="PSUM") as psum,
            tc.tile_pool(name="wstage", bufs=1) as wstage,
        ):
            # ---- persistent weights (fp32r, converted on-chip) ----
            # wih k-chunks: [128,900], [128,900], [45,900] (incl bias row, base 0)
            # whh k-chunks: [128,900], [128,900], [44,900 @ base 64]
            wih_c = [
                singles.tile([128, GATE3], F32R, tag="wih0", name="wih0"),
                singles.tile([128, GATE3], F32R, tag="wih1", name="wih1"),
                singles.tile([C2 + 1, GATE3], F32R, tag="wih2", name="wih2"),
            ]
            whh_c = [
                singles.tile([128, GATE3], F32R, tag="whh0", name="whh0"),
                singles.tile([128, GATE3], F32R, tag="whh1", name="whh1"),
                singles.tile([C2, GATE3], F32R, tag="whh2", name="whh2"),
            ]
            # wstage stays open for the whole program: closing it early would let
            # the hp pool reuse its addresses and chain >2 DMA-queue WAR waits
            # onto one DMA (DMA pseudo-instructions have a tiny wait-slot budget)
            if True:
                for k, (c0, cl) in enumerate(HCHUNK):
                    if k < 2:
                        st = wstage.tile(
                            [128, GATE3], f32, tag=f"wsti{k}", name=f"wsti{k}"
                        )
                        nc.sync.dma_start(st[0:cl, :], wih_d[c0 : c0 + cl, :])
                        nc.vector.tensor_copy(wih_c[k][0:cl, :], st[0:cl, :])
                    else:
                        st = wstage.tile(
                            [C2 + 1, GATE3], f32, tag="wsti2", name="wsti2"
                        )
                        nc.sync.dma_start(st[:, :], wih_d[c0 : c0 + cl + 1, :])
                        nc.vector.tensor_copy(wih_c[2][:, :], st[:, :])
                for k, (c0, cl) in enumerate(HCHUNK):
                    st = wstage.tile([128, GATE3], f32, tag=f"wsth{k}", name=f"wsth{k}")
                    nc.sync.dma_start(st[0:cl, :], whh_d[c0 : c0 + cl, :])
                    nc.vector.tensor_copy(whh_c[k][:, :], st[0:cl, :])
            biasb = singles.tile([128, 6], f32, tag="biasb")
            nc.sync.dma_start(biasb[:, :], biasq_d[:, :])

            # ---- persistent state (ping-pong, fp32r) ----
            # chunks 0/1 merged: hq [128, 2, NB]; chunk 2: [44, NB] at base 0,
            # plus a DMA-shifted copy at partitions 64:108 (h2s) so the k2
            # gh matmul can row-pack into the upper PE quadrant (fp32r matmuls
            # must keep their PSUM output at partition 0, so the elementwise
            # pipeline stays at base 0).
            hk = [
                [singles.tile([128, NB], F32R, tag=f"hk{p}_{k}", name=f"hk{p}_{k}")
                 for k in range(2)]
                for p in range(2)
            ]
            h2 = [singles.tile([C2, NB], F32R, tag=f"h2_{p}", name=f"h2_{p}") for p in range(2)]

            def hrhs(p, k):
                return hk[p][k][:, :] if k < 2 else h2[p]

            # ---- prologue: load host-computed hpool as h0 (pp=0) ----
            hp0 = wstage.tile([128, 2, NB], f32, tag="hp0", name="hp0")
            if plain_dma:
                nc.sync.dma_start(hp0[:, 0, :], hpool_d[0:128, :])
                nc.sync.dma_start(hp0[:, 1, :], hpool_d[128:256, :])
            else:
                nc.sync.dma_start(
                    hp0[:, :, :], hpool_d[0:256, :].rearrange("(c p) b -> p c b", c=2)
                )
            hp2 = wstage.tile([C2, NB], f32, tag="hp2", name="hp2")
            nc.sync.dma_start(hp2[:, :], hpool_d[256:300, :])
            nc.vector.tensor_copy(hk[0][0][:, :], hp0[:, 0, :])
            nc.vector.tensor_copy(hk[0][1][:, :], hp0[:, 1, :])
            nc.vector.tensor_copy(h2[0][:, :], hp2[:, :])

            # ---- main recurrence ----
            # (g, col0, gl, pbase); the tail gate-tile goes first so its
            # (longest) elementwise chain overlaps the other tiles' matmuls
            GC = [(2, 256, C2, 0), (0, 0, 128, 0), (1, 128, 128, 0)]
            pp = 0
            for s in range(L):
                # x: merged chunks 0/1 + tail chunk (45 rows incl ones row);
                # DMA into fp32 staging, relu writes the fp32r matmul operand
                xsq = xpool.tile([128, 2, NB], f32, tag="xsq")
                if plain_dma:
                    nc.sync.dma_start(xsq[:, 0, :], xpad_d[0:128, s, :])
                    nc.sync.dma_start(xsq[:, 1, :], xpad_d[128:256, s, :])
                else:
                    nc.sync.dma_start(
                        xsq[:, :, :],
                        xpad_d[0:256, s, :].rearrange("(c p) b -> p c b", c=2),
                    )
                xs2 = xpool.tile([C2 + 1, NB], f32, tag="xs2")
                nc.sync.dma_start(xs2[:, :], xpad_d[256:301, s, :])
                xk = [
                    xpool.tile([128, NB], F32R, tag=f"xk{c}", name=f"xk{c}")
                    for c in range(2)
                ]
                x2 = xpool.tile([C2 + 1, NB], F32R, tag="x2")
                for c in range(2):
                    nc.scalar.activation(
                        xk[c][:, :], xsq[:, c, :], AF.Relu, bias=biasb[:, c : c + 1]
                    )
                nc.scalar.activation(
                    x2[:, :], xs2[:, :], AF.Relu, bias=biasb[0 : C2 + 1, 2:3]
                )

                def xrhs(k):
                    return xk[k][:, :] if k < 2 else x2[:, :]

                def hrhs2(p, k):
                    return hrhs(p, k)

                PS = max(NB, 512)  # bank-sized gate stride (512 fp32 = 1 bank)
                for g, g0, gl, pb in [e for e in GC if only_g is None or e[0] == only_g]:
                    ps_shape = [pb + gl, 2, PS]
                    prz = psum.tile(ps_shape, f32, tag="prz")
                    pn = psum.tile(ps_shape, f32, tag="pn")
                    o_r = prz[pb : pb + gl, 0, 0:NB]
                    o_z = prz[pb : pb + gl, 1, 0:NB]
                    o_xn = pn[pb : pb + gl, 0, 0:NB]
                    o_gn = pn[pb : pb + gl, 1, 0:NB]
                    wcol_r = g0
                    wcol_z = H + g0
                    wcol_n = 2 * H + g0

                    def mm(out_ap, w_tile_ap, rhs_ap, start, stop, row):
                        tp = (row, pb) if (row or pb) else None
                        nc.tensor.matmul(
                            out_ap, w_tile_ap, rhs_ap,
                            start=start, stop=stop, tile_position=tp,
                        )

                    if body_level < 2:
                        # debug: bypass matmuls, copy x into state slot
                        hnew = h2[pp ^ 1] if g == 2 else hk[pp ^ 1][g][:, :]
                        src = xrhs(2)[0:gl, :] if g == 2 else xrhs(g)
                        nc.vector.tensor_copy(hnew, src.bitcast(f32))
                        continue
                    # phase X: x-dependent k0/k1 matmuls
                    for k in range(2):
                        mm(o_r, wih_c[k][:, wcol_r : wcol_r + gl], xrhs(k), k == 0, False, 0)
                        mm(o_z, wih_c[k][:, wcol_z : wcol_z + gl], xrhs(k), k == 0, False, 0)
                        mm(o_xn, wih_c[k][:, wcol_n : wcol_n + gl], xrhs(k), k == 0, False, 0)
                    # phase H: h-dependent k0/k1 matmuls
                    for k in range(2):
                        mm(o_r, whh_c[k][:, wcol_r : wcol_r + gl], hrhs2(pp, k), False, False, 0)
                        mm(o_z, whh_c[k][:, wcol_z : wcol_z + gl], hrhs2(pp, k), False, False, 0)
                        mm(o_gn, whh_c[k][:, wcol_n : wcol_n + gl], hrhs2(pp, k), k == 0, False, 0)
                    # phase P: k2 tail matmuls (all at base 0; row-packing the
                    # tail into the upper PE quadrant faults when full-K
                    # matmuls are still in flight)
                    mm(o_r, whh_c[2][:, wcol_r : wcol_r + gl], hrhs(pp, 2), False, False, 0)
                    mm(o_r, wih_c[2][:, wcol_r : wcol_r + gl], xrhs(2), False, True, 0)
                    mm(o_z, whh_c[2][:, wcol_z : wcol_z + gl], hrhs(pp, 2), False, False, 0)
                    mm(o_z, wih_c[2][:, wcol_z : wcol_z + gl], xrhs(2), False, True, 0)
                    mm(o_gn, whh_c[2][:, wcol_n : wcol_n + gl], hrhs(pp, 2), False, True, 0)
                    mm(o_xn, wih_c[2][:, wcol_n : wcol_n + gl], xrhs(2), False, True, 0)

                    # elementwise (chunk-2 pipeline lives at partitions 64:108)
                    rz = ew.tile([pb + gl, 2, NB], f32, tag="rz")
                    if body_level == 25:  # debug: DVE copy instead of ACT sigmoid
                        nc.vector.tensor_copy(
                            rz[pb : pb + gl, :, :], prz[pb : pb + gl, :, 0:NB]
                        )
                    elif split_sigma:
                        for gi in range(2):
                            nc.scalar.activation(
                                rz[pb : pb + gl, gi, :], prz[pb : pb + gl, gi, 0:NB],
                                AF.Sigmoid,
                            )
                    else:
                        nc.scalar.activation(
                            rz[pb : pb + gl, :, :], prz[pb : pb + gl, :, 0:NB], AF.Sigmoid
                        )
                    if body_level < 3 or body_level == 25:
                        hnew = h2[pp ^ 1] if g == 2 else hk[pp ^ 1][g][:, :]
                        nc.vector.tensor_copy(hnew, rz[pb : pb + gl, 0, :])
                        continue
                    tn1 = ew.tile([pb + gl, NB], f32, tag="tn1")
                    nc.vector.scalar_tensor_tensor(
                        out=tn1[pb : pb + gl, :], in0=o_gn,
                        scalar=biasb[pb : pb + gl, 3 + g : 4 + g],
                        in1=rz[pb : pb + gl, 0, :], op0=OP.add, op1=OP.mult,
                    )
                    tn2 = ew.tile([pb + gl, NB], f32, tag="tn2")
                    nc.vector.tensor_add(tn2[pb : pb + gl, :], tn1[pb : pb + gl, :], o_xn)
                    nn = ew.tile([pb + gl, NB], f32, tag="nn")
                    nc.scalar.activation(nn[pb : pb + gl, :], tn2[pb : pb + gl, :], AF.Tanh)
                    if body_level < 4:
                        hnew = h2[pp ^ 1] if g == 2 else hk[pp ^ 1][g][:, :]
                        nc.vector.tensor_copy(hnew, nn[pb : pb + gl, :])
                        continue
                    hold = h2[pp] if g == 2 else hk[pp][g][:, :]
                    t3 = ew.tile([pb + gl, NB], f32, tag="t3")
                    nc.gpsimd.tensor_sub(
                        t3[pb : pb + gl, :], hold.bitcast(f32), nn[pb : pb + gl, :]
                    )
                    t4 = ew.tile([pb + gl, NB], f32, tag="t4")
                    nc.gpsimd.tensor_mul(
                        t4[pb : pb + gl, :], rz[pb : pb + gl, 1, :], t3[pb : pb + gl, :]
                    )
                    hnew = h2[pp ^ 1] if g == 2 else hk[pp ^ 1][g][:, :]
                    nc.vector.tensor_add(hnew, nn[pb : pb + gl, :], t4[pb : pb + gl, :])
                # y out: merged chunks 0/1, then tail
                nc.sync.dma_start(y_d[0:128, s, :], hk[pp ^ 1][0][:, :].bitcast(f32))
                nc.sync.dma_start(y_d[128:256, s, :], hk[pp ^ 1][1][:, :].bitcast(f32))
                nc.sync.dma_start(y_d[256:300, s, :], h2[pp ^ 1].bitcast(f32))
                pp ^= 1

    nc.compile()
    nc.finalize()
    return nc


def _get_program(NB, L):
    key = (NB, L)
    if key not in _BUILD_CACHE:
        _BUILD_CACHE[key] = _build_program(NB, L)
    return _BUILD_CACHE[key]


def _pack_core(node, starts, sizes, L, rev):
    """Build xpad [H+1, L, NB] fp32 (fill NEG_FILL, ones row at H)."""
    NB = starts.shape[0]
    N = node.shape[0]
    li = np.arange(L)
    idx = np.clip(starts[:, None] + li[None, :], 0, N - 1)
    g = node[idx]  # [NB, L, H]
    mask = li[None, :] < sizes[:, None]
    g[~mask] = NEG_FILL
    if rev:
        g = g[:, ::-1, :]
    out = np.empty((H + 1, L, NB), np.float32)
    out[:H] = g.transpose(2, 1, 0)
    out[H] = 1.0
    return out


def _aug_weights(W_ih, W_hh, b_ih, b_hh):
    # x-side bias row: b_ih everywhere + b_hh on the r,z gates (their gh part
    # accumulates into the same PSUM); b_hh_n is applied separately on-chip.
    xbias = b_ih + np.concatenate([b_hh[: 2 * H], np.zeros(H, np.float32)])
    wih = np.concatenate([W_ih.T, xbias[None, :]], axis=0)  # [H+1, 900]
    whh = W_hh.T  # [H, 900]
    return np.ascontiguousarray(wih, dtype=np.float32), np.ascontiguousarray(
        whh, dtype=np.float32
    )


def prepare_in_maps(np_inputs):
    """Host-side sharding/packing: full inputs -> program + per-core in_maps."""
    out = _prepare(**np_inputs)
    return out


def _prepare(
    node, bias, W_ih_f, W_hh_f, b_ih_f, b_hh_f,
    W_ih_b, W_hh_b, b_ih_b, b_hh_b, starts, sizes, seg_id, offset,
):
    node = np.asarray(node, dtype=np.float32)
    bias = np.asarray(bias, dtype=np.float32)
    starts = np.asarray(starts, dtype=np.int64)
    sizes = np.asarray(sizes, dtype=np.int64)
    seg_id = np.asarray(seg_id, dtype=np.int64)
    offset = np.asarray(offset, dtype=np.int64)
    weights = {
        "f": [np.asarray(a, np.float32) for a in (W_ih_f, W_hh_f, b_ih_f, b_hh_f)],
        "b": [np.asarray(a, np.float32) for a in (W_ih_b, W_hh_b, b_ih_b, b_hh_b)],
    }

    N = node.shape[0]
    B = starts.shape[0]
    L = 64
    NSHARD = 4
    NBSH = B // NSHARD  # graphs per shard (512)

    nc = _get_program(NBSH, L)

    wih_f, whh_f = _aug_weights(*weights["f"])
    wih_b, whh_b = _aug_weights(*weights["b"])

    def _biasq(b_hh):
        q = np.zeros((128, 6), np.float32)
        for k, (c0, cl) in enumerate(HCHUNK):
            q[0:cl, k] = bias[c0 : c0 + cl]
            q[0:cl, 3 + k] = b_hh[2 * H + c0 : 2 * H + c0 + cl]
        return q

    biasq_f = _biasq(weights["f"][3])
    biasq_b = _biasq(weights["b"][3])

    in_maps = []
    shard_rows = []
    hpools = []
    for c in range(NSHARD):
        g0 = c * NBSH
        s_starts = starts[g0 : g0 + NBSH]
        s_sizes = sizes[g0 : g0 + NBSH]
        r0 = int(s_starts[0])
        r1 = int(starts[g0 + NBSH]) if g0 + NBSH < B else N
        shard_rows.append((g0, r0, r1))
        hp = np.maximum.reduceat(node[r0:r1], (s_starts - r0).astype(np.intp), axis=0)
        hpools.append(np.ascontiguousarray(hp.T))  # [H, NB]
        in_maps.append(
            {
                "xpad": _pack_core(node, s_starts, s_sizes, L, rev=False),
                "wih": wih_f, "whh": whh_f, "biasq": biasq_f, "hpool": hpools[c],
            }
        )
    for c in range(NSHARD):
        g0 = c * NBSH
        in_maps.append(
            {
                "xpad": _pack_core(
                    node, starts[g0 : g0 + NBSH], sizes[g0 : g0 + NBSH], L, rev=True
                ),
                "wih": wih_b, "whh": whh_b, "biasq": biasq_b, "hpool": hpools[c],
            }
        )

    return {
        "nc": nc,
        "in_maps": in_maps,
        "shard_rows": shard_rows,
        "meta": (node, bias, seg_id, offset, N, NBSH, NSHARD),
    }


def kernel(**np_inputs):
    from concourse.bass_utils import run_bass_kernel_spmd

    prep = _prepare(**{k: np.asarray(v) for k, v in np_inputs.items()})
    nc, in_maps = prep["nc"], prep["in_maps"]
    node, bias, seg_id, offset, N, NBSH, NSHARD = prep["meta"]

    trace = bool(os.environ.get("GRU_KERNEL_TRACE"))
    res = run_bass_kernel_spmd(nc, in_maps, list(range(len(in_maps))), trace=trace)
    kernel.last_exec_time_ns = res.exec_time_ns
    results = res.results

    out = np.empty((N + 1, 2 * H), np.float32)
    head = np.maximum(node[0] + bias, 0.0)
    out[0, :H] = head
    out[0, H:] = head
    for c in range(NSHARD):
        g0, r0, r1 = prep["shard_rows"][c]
        y_f = results[c]["y"]  # [H, L, NB]
        y_b = results[NSHARD + c]["y"][:, ::-1, :]
        bl = seg_id[r0:r1] - g0
        off = offset[r0:r1]
        cols = off * NBSH + bl
        out[1 + r0 : 1 + r1, 0:H] = y_f.reshape(H, -1)[:, cols].T
        out[1 + r0 : 1 + r1, H : 2 * H] = y_b.reshape(H, -1)[:, cols].T
    return out


kernel.last_exec_time_ns = None


# revision 42
# speedup vs baseline: 51.8931x; 51.8931x over previous
"""Trainium2 Bass kernel for nn_BatchGRUNet (bidirectional GRU over ragged graph batch).

Contract: kernel(**inputs) takes the FULL unsharded inputs (as produced by
reference.setup_inputs()) and returns the FULL [N+1, 2H] output.

Strategy (8 NeuronCores, SPMD):
  - 2048 graphs are split into 4 shards of 512 graphs.
  - Cores 0..3 run the FORWARD GRU for shards 0..3; cores 4..7 run the
    BACKWARD GRU for the same shards, fed with time-reversed packed input
    (a backward scan == forward scan on reversed time), so all 8 cores run
    the identical single-direction program on different data.
  - Host packs the ragged node rows into a padded, transposed layout
    xpad[h, t, b] (fill -1e30) per core and precomputes hpool (segment max,
    the GRU initial state); the device computes message = relu(node + bias),
    the 64-step GRU recurrence (fp32 elementwise, matmuls in float32r/tf32),
    and writes padded-transposed outputs y[h, t, b]; the host unpads back to
    flat node order.

Layout on device (per core, per step):
  - state h^T [300, 512] lives H-on-partitions in 3 chunks (128/128/44).
  - i = x@W_ih^T + b_ih + h@W_hh^T + b_hh is accumulated per gate directly
    in PSUM (x-side biases folded in via an augmented ones-row on x and a
    bias row on wih; b_hh_n applied as a per-partition scalar on-chip).
  - r/z: sigmoid straight out of PSUM; the n-gate keeps its xp and gh parts
    separate (torch GRU: n = tanh(i_n + r * h_n)).
  - All float32r operands are written CONTIGUOUSLY by their producing engine
    op: strided fp32r engine writes fault the exec unit (found empirically),
    and fp32r matmuls must keep PSUM output at partition base 0.
"""

import os

import numpy as np

H = 300
GATE3 = 3 * H  # 900
HCHUNK = [(0, 128), (128, 128), (256, 44)]  # (start, len) partition chunks of H
NEG_FILL = np.float32(-1.0e30)

_BUILD_CACHE = {}


def _build_program(NB, L, no_pack=False, split_sigma=False, plain_dma=False, body_level=4, psum_bufs=2, only_g=None, flat_rhs=False):
    """Build the single-direction GRU Bass program (SPMD across cores).

    NB: per-core batch (number of graphs); L: number of time steps.

    Matmul inputs (x, h, W) are float32r (tf32): the PE streams fp32r at
    full rate when the moving dim >= 256 (vs 4x slower for plain fp32).
    Producers round on write (ACT relu for x, DVE adds/copies for h and W).
    """
    import concourse.bass as bass
    import concourse.tile as tile
    from concourse import bacc, mybir

    f32 = mybir.dt.float32
    F32R = mybir.dt.float32r
    AF = mybir.ActivationFunctionType
    OP = mybir.AluOpType
    X = mybir.AxisListType.X

    nc = bacc.Bacc("TRN2", target_bir_lowering=False)

    # xpad row H (=300) is an all-ones row: the augmented-K input that applies
    # the x-side bias row of wih inside the matmul accumulation.
    xpad_d = nc.dram_tensor("xpad", [H + 1, L, NB], f32, kind="ExternalInput")
    wih_d = nc.dram_tensor("wih", [H + 1, GATE3], f32, kind="ExternalInput")
    whh_d = nc.dram_tensor("whh", [H, GATE3], f32, kind="ExternalInput")
    # biasq cols 0..2: relu bias (node bias) per H-chunk; cols 3..5: b_hh_n chunks
    biasq_d = nc.dram_tensor("biasq", [128, 6], f32, kind="ExternalInput")
    hpool_d = nc.dram_tensor("hpool", [H, NB], f32, kind="ExternalInput")
    y_d = nc.dram_tensor("y", [H, L, NB], f32, kind="ExternalOutput")

    C2 = 44  # tail chunk rows
    B2 = 64  # partition base of the tail chunk (state/weights/gate pipeline)

    with tile.TileContext(nc) as tc:
        with (
            tc.tile_pool(name="singles", bufs=1) as singles,
            tc.tile_pool(name="xpool", bufs=2) as xpool,
            tc.tile_pool(name="ew", bufs=2) as ew,
            tc.tile_pool(name="psum", bufs=psum_bufs, space="PSUM") as psum,
            tc.tile_pool(name="wstage", bufs=1) as wstage,
        ):
            # ---- persistent weights (fp32r, converted on-chip) ----
            # wih k-chunks: [128,900], [128,900], [45,900] (incl bias row, base 0)
            # whh k-chunks: [128,900], [128,900], [44,900 @ base 64]
            wih_c = [
                singles.tile([128, GATE3], F32R, tag="wih0", name="wih0"),
                singles.tile([128, GATE3], F32R, tag="wih1", name="wih1"),
                singles.tile([C2 + 1, GATE3], F32R, tag="wih2", name="wih2"),
            ]
            whh_c = [
                singles.tile([128, GATE3], F32R, tag="whh0", name="whh0"),
                singles.tile([128, GATE3], F32R, tag="whh1", name="whh1"),
                singles.tile([C2, GATE3], F32R, tag="whh2", name="whh2"),
            ]
            # wstage stays open for the whole program: closing it early would let
            # the hp pool reuse its addresses and chain >2 DMA-queue WAR waits
            # onto one DMA (DMA pseudo-instructions have a tiny wait-slot budget)
            if True:
                for k, (c0, cl) in enumerate(HCHUNK):
                    if k < 2:
                        st = wstage.tile(
                            [128, GATE3], f32, tag=f"wsti{k}", name=f"wsti{k}"
                        )
                        nc.sync.dma_start(st[0:cl, :], wih_d[c0 : c0 + cl, :])
                        nc.vector.tensor_copy(wih_c[k][0:cl, :], st[0:cl, :])
                    else:
                        st = wstage.tile(
                            [C2 + 1, GATE3], f32, tag="wsti2", name="wsti2"
                        )
                        nc.sync.dma_start(st[:, :], wih_d[c0 : c0 + cl + 1, :])
                        nc.vector.tensor_copy(wih_c[2][:, :], st[:, :])
                for k, (c0, cl) in enumerate(HCHUNK):
                    st = wstage.tile([128, GATE3], f32, tag=f"wsth{k}", name=f"wsth{k}")
                    nc.sync.dma_start(st[0:cl, :], whh_d[c0 : c0 + cl, :])
                    nc.vector.tensor_copy(whh_c[k][:, :], st[0:cl, :])
            biasb = singles.tile([128, 6], f32, tag="biasb")
            nc.sync.dma_start(biasb[:, :], biasq_d[:, :])

            # ---- persistent state (ping-pong, fp32r, contiguous tiles) ----
            hk = [
                [singles.tile([128, NB], F32R, tag=f"hk{p}_{k}", name=f"hk{p}_{k}")
                 for k in range(2)]
                for p in range(2)
            ]
            h2 = [singles.tile([C2, NB], F32R, tag=f"h2_{p}", name=f"h2_{p}") for p in range(2)]

            def hrhs(p, k):
                return hk[p][k][:, :] if k < 2 else h2[p]

            # ---- prologue: load host-computed hpool as h0 (pp=0) ----
            hp0 = wstage.tile([128, 2, NB], f32, tag="hp0", name="hp0")
            if plain_dma:
                nc.sync.dma_start(hp0[:, 0, :], hpool_d[0:128, :])
                nc.sync.dma_start(hp0[:, 1, :], hpool_d[128:256, :])
            else:
                nc.sync.dma_start(
                    hp0[:, :, :], hpool_d[0:256, :].rearrange("(c p) b -> p c b", c=2)
                )
            hp2 = wstage.tile([C2, NB], f32, tag="hp2", name="hp2")
            nc.sync.dma_start(hp2[:, :], hpool_d[256:300, :])
            nc.vector.tensor_copy(hk[0][0][:, :], hp0[:, 0, :])
            nc.vector.tensor_copy(hk[0][1][:, :], hp0[:, 1, :])
            nc.vector.tensor_copy(h2[0][:, :], hp2[:, :])

            # ---- main recurrence ----
            # (g, col0, gl, pbase); the tail gate-tile goes first so its
            # (longest) elementwise chain overlaps the other tiles' matmuls
            GC = [(2, 256, C2, 0), (0, 0, 128, 0), (1, 128, 128, 0)]
            pp = 0
            for s in range(L):
                # x: merged chunks 0/1 + tail chunk (45 rows incl ones row);
                # DMA into fp32 staging, relu writes the fp32r matmul operand
                xsq = xpool.tile([128, 2, NB], f32, tag="xsq")
                if plain_dma:
                    nc.sync.dma_start(xsq[:, 0, :], xpad_d[0:128, s, :])
                    nc.sync.dma_start(xsq[:, 1, :], xpad_d[128:256, s, :])
                else:
                    nc.sync.dma_start(
                        xsq[:, :, :],
                        xpad_d[0:256, s, :].rearrange("(c p) b -> p c b", c=2),
                    )
                xs2 = xpool.tile([C2 + 1, NB], f32, tag="xs2")
                nc.sync.dma_start(xs2[:, :], xpad_d[256:301, s, :])
                xk = [
                    xpool.tile([128, NB], F32R, tag=f"xk{c}", name=f"xk{c}")
                    for c in range(2)
                ]
                x2 = xpool.tile([C2 + 1, NB], F32R, tag="x2")
                for c in range(2):
                    nc.scalar.activation(
                        xk[c][:, :], xsq[:, c, :], AF.Relu, bias=biasb[:, c : c + 1]
                    )
                nc.scalar.activation(
                    x2[:, :], xs2[:, :], AF.Relu, bias=biasb[0 : C2 + 1, 2:3]
                )

                def xrhs(k):
                    return xk[k][:, :] if k < 2 else x2[:, :]

                def hrhs2(p, k):
                    return hrhs(p, k)

                PS = max(NB, 512)  # bank-sized gate stride (512 fp32 = 1 bank)
                for g, g0, gl, pb in [e for e in GC if only_g is None or e[0] == only_g]:
                    ps_shape = [pb + gl, 2, PS]
                    prz = psum.tile(ps_shape, f32, tag="prz")
                    pn = psum.tile(ps_shape, f32, tag="pn")
                    o_r = prz[pb : pb + gl, 0, 0:NB]
                    o_z = prz[pb : pb + gl, 1, 0:NB]
                    o_xn = pn[pb : pb + gl, 0, 0:NB]
                    o_gn = pn[pb : pb + gl, 1, 0:NB]
                    wcol_r = g0
                    wcol_z = H + g0
                    wcol_n = 2 * H + g0

                    def mm(out_ap, w_tile_ap, rhs_ap, start, stop, row):
                        tp = (row, pb) if (row or pb) else None
                        nc.tensor.matmul(
                            out_ap, w_tile_ap, rhs_ap,
                            start=start, stop=stop, tile_position=tp,
                        )

                    if body_level < 2:
                        # debug: bypass matmuls, copy x into state slot
                        hnew = h2[pp ^ 1] if g == 2 else hk[pp ^ 1][g][:, :]
                        src = xrhs(2)[0:gl, :] if g == 2 else xrhs(g)
                        nc.vector.tensor_copy(hnew, src.bitcast(f32))
                        continue
                    # phase X: x-dependent k0/k1 matmuls
                    for k in range(2):
                        mm(o_r, wih_c[k][:, wcol_r : wcol_r + gl], xrhs(k), k == 0, False, 0)
                        mm(o_z, wih_c[k][:, wcol_z : wcol_z + gl], xrhs(k), k == 0, False, 0)
                        mm(o_xn, wih_c[k][:, wcol_n : wcol_n + gl], xrhs(k), k == 0, False, 0)
                    # phase H: h-dependent k0/k1 matmuls
                    for k in range(2):
                        mm(o_r, whh_c[k][:, wcol_r : wcol_r + gl], hrhs2(pp, k), False, False, 0)
                        mm(o_z, whh_c[k][:, wcol_z : wcol_z + gl], hrhs2(pp, k), False, False, 0)
                        mm(o_gn, whh_c[k][:, wcol_n : wcol_n + gl], hrhs2(pp, k), k == 0, False, 0)
                    # phase P: k2 tail matmuls (all at base 0; row-packing the
                    # tail into the upper PE quadrant faults when full-K
                    # matmuls are still in flight)
                    mm(o_r, whh_c[2][:, wcol_r : wcol_r + gl], hrhs(pp, 2), False, False, 0)
                    mm(o_r, wih_c[2][:, wcol_r : wcol_r + gl], xrhs(2), False, True, 0)
                    mm(o_z, whh_c[2][:, wcol_z : wcol_z + gl], hrhs(pp, 2), False, False, 0)
                    mm(o_z, wih_c[2][:, wcol_z : wcol_z + gl], xrhs(2), False, True, 0)
                    mm(o_gn, whh_c[2][:, wcol_n : wcol_n + gl], hrhs(pp, 2), False, True, 0)
                    mm(o_xn, wih_c[2][:, wcol_n : wcol_n + gl], xrhs(2), False, True, 0)

                    # elementwise (chunk-2 pipeline lives at partitions 64:108)
                    rz = ew.tile([pb + gl, 2, NB], f32, tag="rz")
                    if body_level == 25:  # debug: DVE copy instead of ACT sigmoid
                        nc.vector.tensor_copy(
                            rz[pb : pb + gl, :, :], prz[pb : pb + gl, :, 0:NB]
                        )
                    elif split_sigma:
                        for gi in range(2):
                            nc.scalar.activation(
                                rz[pb : pb + gl, gi, :], prz[pb : pb + gl, gi, 0:NB],
                                AF.Sigmoid,
                            )
                    else:
                        nc.scalar.activation(
                            rz[pb : pb + gl, :, :], prz[pb : pb + gl, :, 0:NB], AF.Sigmoid
                        )
                    if body_level < 3 or body_level == 25:
                        hnew = h2[pp ^ 1] if g == 2 else hk[pp ^ 1][g][:, :]
                        nc.vector.tensor_copy(hnew, rz[pb : pb + gl, 0, :])
                        continue
                    tn1 = ew.tile([pb + gl, NB], f32, tag="tn1")
                    nc.vector.scalar_tensor_tensor(
                        out=tn1[pb : pb + gl, :], in0=o_gn,
                        scalar=biasb[pb : pb + gl, 3 + g : 4 + g],
                        in1=rz[pb : pb + gl, 0, :], op0=OP.add, op1=OP.mult,
                    )
                    tn2 = ew.tile([pb + gl, NB], f32, tag="tn2")
                    nc.vector.tensor_add(tn2[pb : pb + gl, :], tn1[pb : pb + gl, :], o_xn)
                    nn = ew.tile([pb + gl, NB], f32, tag="nn")
                    nc.scalar.activation(nn[pb : pb + gl, :], tn2[pb : pb + gl, :], AF.Tanh)
                    if body_level < 4:
                        hnew = h2[pp ^ 1] if g == 2 else hk[pp ^ 1][g][:, :]
                        nc.vector.tensor_copy(hnew, nn[pb : pb + gl, :])
                        continue
                    hold = h2[pp] if g == 2 else hk[pp][g][:, :]
                    t3 = ew.tile([pb + gl, NB], f32, tag="t3")
                    nc.gpsimd.tensor_sub(
                        t3[pb : pb + gl, :], hold.bitcast(f32), nn[pb : pb + gl, :]
                    )
                    t4 = ew.tile([pb + gl, NB], f32, tag="t4")
                    nc.gpsimd.tensor_mul(
                        t4[pb : pb + gl, :], rz[pb : pb + gl, 1, :], t3[pb : pb + gl, :]
                    )
                    hnew = h2[pp ^ 1] if g == 2 else hk[pp ^ 1][g][:, :]
                    nc.vector.tensor_add(hnew, nn[pb : pb + gl, :], t4[pb : pb + gl, :])
                # y out: merged chunks 0/1, then tail
                nc.sync.dma_start(y_d[0:128, s, :], hk[pp ^ 1][0][:, :].bitcast(f32))
                nc.sync.dma_start(y_d[128:256, s, :], hk[pp ^ 1][1][:, :].bitcast(f32))
                nc.sync.dma_start(y_d[256:300, s, :], h2[pp ^ 1].bitcast(f32))
                pp ^= 1

    nc.compile()
    nc.finalize()
    return nc


def _get_program(NB, L):
    key = (NB, L)
    if key not in _BUILD_CACHE:
        _BUILD_CACHE[key] = _build_program(NB, L)
    return _BUILD_CACHE[key]


def _pack_core(node, starts, sizes, L, rev):
    """Build xpad [H+1, L, NB] fp32 (fill NEG_FILL, ones row at H)."""
    NB = starts.shape[0]
    N = node.shape[0]
    li = np.arange(L)
    idx = np.clip(starts[:, None] + li[None, :], 0, N - 1)
    g = node[idx]  # [NB, L, H]
    mask = li[None, :] < sizes[:, None]
    g[~mask] = NEG_FILL
    if rev:
        g = g[:, ::-1, :]
    out = np.empty((H + 1, L, NB), np.float32)
    out[:H] = g.transpose(2, 1, 0)
    out[H] = 1.0
    return out


def _aug_weights(W_ih, W_hh, b_ih, b_hh):
    # x-side bias row: b_ih everywhere + b_hh on the r,z gates (their gh part
    # accumulates into the same PSUM); b_hh_n is applied separately on-chip.
    xbias = b_ih + np.concatenate([b_hh[: 2 * H], np.zeros(H, np.float32)])
    wih = np.concatenate([W_ih.T, xbias[None, :]], axis=0)  # [H+1, 900]
    whh = W_hh.T  # [H, 900]
    return np.ascontiguousarray(wih, dtype=np.float32), np.ascontiguousarray(
        whh, dtype=np.float32
    )


def prepare_in_maps(np_inputs):
    """Host-side sharding/packing: full inputs -> program + per-core in_maps."""
    out = _prepare(**np_inputs)
    return out


def _prepare(
    node, bias, W_ih_f, W_hh_f, b_ih_f, b_hh_f,
    W_ih_b, W_hh_b, b_ih_b, b_hh_b, starts, sizes, seg_id, offset,
):
    node = np.asarray(node, dtype=np.float32)
    bias = np.asarray(bias, dtype=np.float32)
    starts = np.asarray(starts, dtype=np.int64)
    sizes = np.asarray(sizes, dtype=np.int64)
    seg_id = np.asarray(seg_id, dtype=np.int64)
    offset = np.asarray(offset, dtype=np.int64)
    weights = {
        "f": [np.asarray(a, np.float32) for a in (W_ih_f, W_hh_f, b_ih_f, b_hh_f)],
        "b": [np.asarray(a, np.float32) for a in (W_ih_b, W_hh_b, b_ih_b, b_hh_b)],
    }

    N = node.shape[0]
    B = starts.shape[0]
    L = 64
    NSHARD = 4
    NBSH = B // NSHARD  # graphs per shard (512)

    nc = _get_program(NBSH, L)

    wih_f, whh_f = _aug_weights(*weights["f"])
    wih_b, whh_b = _aug_weights(*weights["b"])

    def _biasq(b_hh):
        q = np.zeros((128, 6), np.float32)
        for k, (c0, cl) in enumerate(HCHUNK):
            q[0:cl, k] = bias[c0 : c0 + cl]
            q[0:cl, 3 + k] = b_hh[2 * H + c0 : 2 * H + c0 + cl]
        return q

    biasq_f = _biasq(weights["f"][3])
    biasq_b = _biasq(weights["b"][3])

    in_maps = []
    shard_rows = []
    hpools = []
    for c in range(NSHARD):
        g0 = c * NBSH
        s_starts = starts[g0 : g0 + NBSH]
        s_sizes = sizes[g0 : g0 + NBSH]
        r0 = int(s_starts[0])
        r1 = int(starts[g0 + NBSH]) if g0 + NBSH < B else N
        shard_rows.append((g0, r0, r1))
        hp = np.maximum.reduceat(node[r0:r1], (s_starts - r0).astype(np.intp), axis=0)
        hpools.append(np.ascontiguousarray(hp.T))  # [H, NB]
        in_maps.append(
            {
                "xpad": _pack_core(node, s_starts, s_sizes, L, rev=False),
                "wih": wih_f, "whh": whh_f, "biasq": biasq_f, "hpool": hpools[c],
            }
        )
    for c in range(NSHARD):
        g0 = c * NBSH
        in_maps.append(
            {
                "xpad": _pack_core(
                    node, starts[g0 : g0 + NBSH], sizes[g0 : g0 + NBSH], L, rev=True
                ),
                "wih": wih_b, "whh": whh_b, "biasq": biasq_b, "hpool": hpools[c],
            }
        )

    return {
        "nc": nc,
        "in_maps": in_maps,
        "shard_rows": shard_rows,
        "meta": (node, bias, seg_id, offset, N, NBSH, NSHARD),
    }


def kernel(**np_inputs):
    from concourse.bass_utils import run_bass_kernel_spmd

    prep = _prepare(**{k: np.asarray(v) for k, v in np_inputs.items()})
    nc, in_maps = prep["nc"], prep["in_maps"]
    node, bias, seg_id, offset, N, NBSH, NSHARD = prep["meta"]

    trace = bool(os.environ.get("GRU_KERNEL_TRACE"))
    res = run_bass_kernel_spmd(nc, in_maps, list(range(len(in_maps))), trace=trace)
    kernel.last_exec_time_ns = res.exec_time_ns
    results = res.results

    out = np.empty((N + 1, 2 * H), np.float32)
    head = np.maximum(node[0] + bias, 0.0)
    out[0, :H] = head
    out[0, H:] = head
    for c in range(NSHARD):
        g0, r0, r1 = prep["shard_rows"][c]
        y_f = results[c]["y"]  # [H, L, NB]
        y_b = results[NSHARD + c]["y"][:, ::-1, :]
        bl = seg_id[r0:r1] - g0
        off = offset[r0:r1]
        cols = off * NBSH + bl
        out[1 + r0 : 1 + r1, 0:H] = y_f.reshape(H, -1)[:, cols].T
        out[1 + r0 : 1 + r1, H : 2 * H] = y_b.reshape(H, -1)[:, cols].T
    return out


kernel.last_exec_time_ns = None
